# revision 19
# baseline (speedup 1.0000x reference)
"""Trainium2 Bass kernel for SimCLR NT-Xent contrastive loss (N=4096, D=512, T=0.5).

v5: symmetry-exploiting variant. Global S = z z^T is symmetric, so each core
computes only its row block [0:1024) x a 5120-column window [0:5120) of its
ROTATED similarity matrix (rotation by 1024*c). Block-distance coverage:
  d0 (diag block) and d4 (antipodal block): every unordered pair appears as
     two computed entries across the fleet -> row-sums only.
  d1..d3: every unordered pair appears exactly once -> row-sum AND column-sum
     (the column-sum credits the transposed row's denominator; merged on host).
Norms are window-local: each core normalizes only the 5120 rows it uses as
columns (rows [0:5120) rotated), so there is no cross-core norm exchange.

Inputs per core (host-staged): xr = rotated rows [0:5120) bf16 row-major,
xt = rotated cols [0:5120) bf16 transposed. fp8e4m3 DoubleRow matmuls on
z8 = 16 * z (exp scale folds 1/256). Denominator parts: row-sums via fused
ACT-exp accumulation; column-sums for d1-d3 via ones-vector matmuls chained
over the 8 m-tiles in PSUM. Host merges rowsums [128,8], colsums [1,3072],
pair-dots [128,8]: loss = (sum log(den - e^2) - 2*sum pos) / 8192.
"""

import numpy as np
import ml_dtypes

for _p in ("/opt/trn_rl_repo", "/root/.axon_site/_ro/trn_rl_repo"):
    try:
        import concourse  # noqa: F401
        break
    except ImportError:
        import sys
        if _p not in sys.path:
            sys.path.insert(0, _p)

import concourse.bass as bass
import concourse.bacc as bacc
import concourse.tile as tile
from concourse import mybir
from concourse.bass_utils import run_bass_kernel_spmd

F32 = mybir.dt.float32
I32 = mybir.dt.int32
BF16 = mybir.dt.bfloat16
FP8 = mybir.dt.float8e4
ALU = mybir.AluOpType
AF = mybir.ActivationFunctionType

N_CORES = 8
BATCH = 4096
DIM = 512
ROWS = 2 * BATCH            # 8192
BLOCK = ROWS // N_CORES     # 1024 rows per core
P = 128                     # partitions
KC = DIM // P               # 4 k-chunks
MT = BLOCK // P             # 8 m-tiles
NCG = 5                     # column sub-groups in the window
CGW = 1024                  # cols per group
WIN = NCG * CGW             # 5120-column window
NW = 512                    # matmul free width
CS_LO, CS_HI = 1, 4         # groups with column-sum credit (d1..d3)
TEMP_SCALE = 2.0            # 1/T
SCALE_UP = 16.0             # fp8 pre-scale; exp scale folds 1/SCALE_UP^2
MAGIC = 0x5F3759DF


def _build_program():
    nc = bacc.Bacc(trn_type="TRN2")
    xr_in = nc.declare_dram_parameter("xr", [WIN, DIM], BF16, isOutput=False)
    xt_in = nc.declare_dram_parameter("xt", [DIM, WIN], BF16, isOutput=False)
    den_out = nc.declare_dram_parameter("den", [P, MT], F32, isOutput=True)
    pos_out = nc.declare_dram_parameter("pos", [P, MT], F32, isOutput=True)
    cs_out = nc.declare_dram_parameter(
        "cs", [1, (CS_HI - CS_LO) * CGW], F32, isOutput=True)

    with tile.TileContext(nc) as tc:
        with tc.tile_pool(name="xg", bufs=3) as xg_pool, \
             tc.tile_pool(name="sq", bufs=3) as sq_pool, \
             tc.tile_pool(name="small", bufs=2) as small_pool, \
             tc.tile_pool(name="ztr", bufs=12) as ztr_pool, \
             tc.tile_pool(name="wide", bufs=1) as wide_pool, \
             tc.tile_pool(name="bcast", bufs=2) as bcast_pool, \
             tc.tile_pool(name="single", bufs=1) as singles, \
             tc.tile_pool(name="escr", bufs=3) as e_pool, \
             tc.tile_pool(name="invd", bufs=2, space="DRAM") as invd_pool, \
             tc.tile_pool(name="mmps", bufs=3, space="PSUM") as mm_psum, \
             tc.tile_pool(name="csps", bufs=1, space="PSUM") as cs_psum:

            n2 = singles.tile([P, NCG * MT], F32, tag="n2")
            inv = singles.tile([P, NCG * MT], F32, tag="inv")
            magic = singles.tile([P, MT], I32, tag="magic")
            nc.vector.memset(magic, MAGIC)
            ones = singles.tile([P, 1], BF16, tag="ones")
            nc.vector.memset(ones, 1.0)
            ones128 = singles.tile([P, P], BF16, tag="ones128")
            nc.vector.memset(ones128, 1.0)
            magic_w = singles.tile([P, CGW], I32, tag="magic_w")
            nc.vector.memset(magic_w, MAGIC)
            accm = singles.tile([P, MT * NCG], F32, tag="accm")
            posraw = singles.tile([P, MT], F32, tag="posraw")
            pos_t = singles.tile([P, MT], F32, tag="pos_t")
            den_t = singles.tile([P, MT], F32, tag="den_t")
            colsb = singles.tile([1, (CS_HI - CS_LO) * CGW], F32, tag="colsb")
            # pinned rows 0-1023 (positive-pair partners for rows 4096-5119)
            xg_pin = singles.tile([P, MT, DIM], BF16, tag="xgpin")
            # scaled fp8 zT tiles, k-interleaved for DoubleRow:
            # zt8[kk][cg][p, i, c] = SCALE_UP * z[col c, d=256*kk+128*i+p]
            zt8 = [[singles.tile([P, 2, CGW], FP8, tag=f"zt8_{kk}_{c}",
                                 name=f"zt8_{kk}_{c}")
                    for c in range(NCG)] for kk in range(2)]

            def prep(cg):
                if cg == 0:
                    # Fast-path bc for the ramp-critical first group: column
                    # norms computed directly in broadcast layout on the PE
                    # (ones^T @ xt^2 duplicates each column sum across all
                    # 128 partitions), then a wide Quake rsqrt -> bc. Avoids
                    # waiting on the row-major load + scatter roundtrip.
                    ztrs = []
                    for k in range(KC):
                        ztr = ztr_pool.tile([P, CGW], BF16, tag="ztr",
                                            name=f"ztr0_{k}")
                        nc.sync.dma_start(
                            out=ztr, in_=xt_in[k * P:(k + 1) * P, 0:CGW])
                        ztrs.append(ztr)
                    sqzs = []
                    for k in range(KC):
                        sqz = ztr_pool.tile([P, CGW], BF16, tag="sqz",
                                            name=f"sqz0_{k}")
                        nc.vector.tensor_mul(out=sqz, in0=ztrs[k], in1=ztrs[k])
                        sqzs.append(sqz)
                    n2p = mm_psum.tile([P, CGW], F32, tag="ps", name="n2p")
                    for n in range(CGW // NW):
                        for k in range(KC):
                            nc.tensor.matmul(
                                n2p[:, n * NW:(n + 1) * NW],
                                lhsT=ones128,
                                rhs=sqzs[k][:, n * NW:(n + 1) * NW],
                                start=(k == 0), stop=(k == KC - 1))
                    shw = wide_pool.tile([P, CGW], I32, tag="wsh")
                    nc.vector.tensor_scalar(
                        out=shw, in0=n2p.bitcast(I32), scalar1=1, scalar2=None,
                        op0=ALU.logical_shift_right)
                    sdw = wide_pool.tile([P, CGW], I32, tag="wsd")
                    nc.vector.scalar_tensor_tensor(
                        out=sdw, in0=magic_w, scalar=0.0, in1=shw,
                        op0=ALU.bypass, op1=ALU.subtract)
                    yw = sdw.bitcast(F32)
                    ivw = wide_pool.tile([P, CGW], F32, tag="wiv")
                    for it in range(2):
                        taw = wide_pool.tile([P, CGW], F32, tag="wta")
                        tbw = wide_pool.tile([P, CGW], F32, tag="wtb")
                        nc.vector.tensor_mul(out=taw, in0=yw, in1=yw)
                        nc.vector.scalar_tensor_tensor(
                            out=tbw, in0=taw, scalar=-0.5, in1=n2p,
                            op0=ALU.mult, op1=ALU.mult)
                        nc.vector.tensor_scalar(
                            out=tbw, in0=tbw, scalar1=1.5, scalar2=None,
                            op0=ALU.add)
                        dst = ivw if it == 1 else yw
                        nc.vector.tensor_mul(out=dst, in0=yw, in1=tbw)
                    bc0 = bcast_pool.tile([P, CGW], BF16, tag="bc")
                    nc.vector.tensor_scalar(
                        out=bc0, in0=ivw, scalar1=SCALE_UP, scalar2=None,
                        op0=ALU.mult)
                    for k in range(KC):
                        nc.vector.tensor_mul(
                            out=zt8[k // 2][0][:, k % 2, :], in0=ztrs[k],
                            in1=bc0)
                # --- load raw rows 1024cg..1024cg+1023, square-accum ---
                r0 = cg * CGW
                xg = xg_pin if cg == 0 else xg_pool.tile(
                    [P, MT, DIM], BF16, tag="xg")
                nc.sync.dma_start(
                    out=xg,
                    in_=xr_in[r0:r0 + CGW, :].rearrange(
                        "(a p) d -> p a d", p=P))
                for a in range(MT):
                    sq = sq_pool.tile([P, DIM], BF16, tag="sq")
                    nc.vector.scalar_tensor_tensor(
                        out=sq, in0=xg[:, a, :], scalar=0.0,
                        in1=xg[:, a, :], op0=ALU.bypass, op1=ALU.mult,
                        accum_out=n2[:, cg * MT + a: cg * MT + a + 1])
                if cg == 4:
                    # positive pairs: rotated rows [0:1024) x [4096:5120)
                    for a in range(MT):
                        psc = sq_pool.tile([P, DIM], BF16, tag="sq")
                        nc.vector.scalar_tensor_tensor(
                            out=psc, in0=xg_pin[:, a, :], scalar=0.0,
                            in1=xg[:, a, :], op0=ALU.bypass, op1=ALU.mult,
                            accum_out=posraw[:, a: a + 1])
                # --- rsqrt on this cg's 8 norms: Quake seed + 2 Newton ---
                sl = n2[:, cg * MT:(cg + 1) * MT]
                isl = inv[:, cg * MT:(cg + 1) * MT]
                sh = small_pool.tile([P, MT], I32, tag="sh")
                nc.vector.tensor_scalar(
                    out=sh, in0=sl.bitcast(I32), scalar1=1, scalar2=None,
                    op0=ALU.logical_shift_right)
                seed = small_pool.tile([P, MT], I32, tag="seed")
                nc.vector.scalar_tensor_tensor(
                    out=seed, in0=magic, scalar=0.0, in1=sh,
                    op0=ALU.bypass, op1=ALU.subtract)
                y = seed.bitcast(F32)
                for it in range(2):
                    ta = small_pool.tile([P, MT], F32, tag="ta")
                    tb = small_pool.tile([P, MT], F32, tag="tb")
                    nc.vector.tensor_mul(out=ta, in0=y, in1=y)
                    nc.vector.scalar_tensor_tensor(
                        out=tb, in0=ta, scalar=-0.5, in1=sl,
                        op0=ALU.mult, op1=ALU.mult)
                    nc.vector.tensor_scalar(
                        out=tb, in0=tb, scalar1=1.5, scalar2=None, op0=ALU.add)
                    dst = isl if it == 1 else y
                    nc.vector.tensor_mul(out=dst, in0=y, in1=tb)
                if cg == 0:
                    return  # bc + scales already done via the PE fast path
                # inv * SCALE_UP as bf16 for the fp8 pre-scale
                iv16 = small_pool.tile([P, 32], BF16, tag="iv16")
                nc.vector.tensor_scalar(
                    out=iv16[:, 0:MT], in0=isl, scalar1=SCALE_UP, scalar2=None,
                    op0=ALU.mult)
                # --- inv -> row-ordered DRAM via DVE 32x32 block transpose,
                # then one contiguous partition-broadcast read back ---
                ivt = small_pool.tile([P, 32], BF16, tag="ivt")
                nc.vector.transpose(out=ivt, in_=iv16)
                invd = invd_pool.tile([CGW], BF16, tag="invd")
                for b in range(4):
                    nc.scalar.dma_start(
                        out=invd.rearrange("(i b j) -> i b j", b=4, j=32)
                                [:, b, :],
                        in_=ivt[32 * b:32 * b + MT, :])
                bc = bcast_pool.tile([P, CGW], BF16, tag="bc")
                nc.scalar.dma_start(
                    out=bc,
                    in_=invd.rearrange("(a f) -> a f", a=1)
                            .partition_broadcast(P))
                # --- load transposed tiles, scale into fp8 DoubleRow layout ---
                for k in range(KC):
                    ztr = ztr_pool.tile([P, CGW], BF16, tag="ztr")
                    nc.sync.dma_start(
                        out=ztr,
                        in_=xt_in[k * P:(k + 1) * P,
                                  cg * CGW:(cg + 1) * CGW])
                    nc.vector.tensor_mul(
                        out=zt8[k // 2][cg][:, k % 2, :], in0=ztr, in1=bc)

            def mmblock(cg):
                do_cs = CS_LO <= cg < CS_HI
                cs = (cs_psum.tile([1, CGW], F32, tag="cs", name=f"cs{cg}")
                      if do_cs else None)
                es = []
                for m in range(MT):
                    ps = mm_psum.tile([P, CGW], F32, tag="ps")
                    for n in range(CGW // NW):
                        for kk in range(2):
                            nc.tensor.matmul(
                                ps[:, n * NW:(n + 1) * NW],
                                lhsT=zt8[kk][0][:, :, m * P:(m + 1) * P],
                                rhs=zt8[kk][cg][:, :, n * NW:(n + 1) * NW],
                                start=(kk == 0), stop=(kk == 1),
                                perf_mode=mybir.MatmulPerfMode.DoubleRow)
                    e_scr = e_pool.tile([P, CGW], BF16, tag="escr")
                    nc.scalar.activation(
                        out=e_scr, in_=ps, func=AF.Exp,
                        scale=TEMP_SCALE / (SCALE_UP * SCALE_UP),
                        accum_out=accm[:, m * NCG + cg: m * NCG + cg + 1])
                    if do_cs:
                        es.append((m, e_scr))
                        # column sums: ones^T @ e, chained over m in PSUM.
                        # Emit one m behind so the PE never waits on exp.
                        if m >= 1:
                            pm, pe = es[m - 1]
                            for n in range(CGW // NW):
                                nc.tensor.matmul(
                                    cs[:, n * NW:(n + 1) * NW],
                                    lhsT=ones, rhs=pe[:, n * NW:(n + 1) * NW],
                                    start=(pm == 0), stop=False,
                                    skip_group_check=True)
                if do_cs:
                    pm, pe = es[MT - 1]
                    for n in range(CGW // NW):
                        nc.tensor.matmul(
                            cs[:, n * NW:(n + 1) * NW],
                            lhsT=ones, rhs=pe[:, n * NW:(n + 1) * NW],
                            start=False, stop=True, skip_group_check=True)
                    nc.vector.tensor_copy(
                        out=colsb[:, (cg - CS_LO) * CGW:
                                  (cg - CS_LO + 1) * CGW], in_=cs)

            prep(0)
            prep(1)
            for cg in range(NCG):
                mmblock(cg)
                if cg + 2 < NCG:
                    prep(cg + 2)

            # --- outputs: raw denominator row-sums + scaled pair-dots ---
            for m in range(MT):
                nc.vector.reduce_sum(
                    out=den_t[:, m:m + 1], in_=accm[:, m * NCG:(m + 1) * NCG],
                    axis=mybir.AxisListType.X)
            nc.vector.tensor_mul(out=pos_t, in0=posraw, in1=inv[:, 0:MT])
            nc.vector.tensor_mul(
                out=pos_t, in0=pos_t, in1=inv[:, 4 * MT:5 * MT])
            nc.sync.dma_start(out=den_out[:, :], in_=den_t)
            nc.sync.dma_start(out=pos_out[:, :], in_=pos_t)
            nc.sync.dma_start(out=cs_out[:, :], in_=colsb)

    nc.finalize()
    return nc


_CACHE = {}


def _run(full: np.ndarray, trace: bool = False, **kwargs):
    """Run the SPMD program on all 8 cores; returns BassKernelResults."""
    if "nc" not in _CACHE:
        _CACHE["nc"] = _build_program()
    nc = _CACHE["nc"]
    xbf = full.astype(ml_dtypes.bfloat16)
    in_maps = []
    for c in range(N_CORES):
        xc = np.ascontiguousarray(np.roll(xbf, -BLOCK * c, axis=0)[:WIN])
        in_maps.append({
            "xr": xc,
            "xt": np.ascontiguousarray(xc.T),
        })
    return run_bass_kernel_spmd(
        nc, in_maps, core_ids=list(range(N_CORES)), trace=trace, **kwargs)


def _merge(results) -> np.ndarray:
    e2 = float(np.exp(2.0))
    den_g = np.zeros(ROWS, dtype=np.float64)
    pos_sum = 0.0
    for c, r in enumerate(results):
        # row-sums: (p, m) -> rotated row 128m+p -> global 1024c+128m+p
        den = r["den"].astype(np.float64)          # [128, 8]
        rows = (1024 * c + 128 * np.arange(MT)[None, :]
                + np.arange(P)[:, None]) % ROWS
        np.add.at(den_g, rows, den)
        # col-sums: j in [0,3072) -> rotated col 1024+j -> global row
        cs = r["cs"].astype(np.float64).ravel()    # [3072]
        cols = (1024 * c + CGW * CS_LO + np.arange(cs.size)) % ROWS
        np.add.at(den_g, cols, cs)
        pos_sum += r["pos"].astype(np.float64).sum()
    loss = (np.log(den_g - e2).sum() - TEMP_SCALE * pos_sum) / (2.0 * BATCH)
    return np.array(loss, dtype=np.float32)


def kernel(emb_i: np.ndarray, emb_j: np.ndarray) -> np.ndarray:
    full = np.concatenate(
        [np.asarray(emb_i, np.float32), np.asarray(emb_j, np.float32)], axis=0)
    return _merge(_run(full).results)


# revision 20
# speedup vs baseline: 1.2134x; 1.2134x over previous
"""Trainium2 Bass kernel for SimCLR NT-Xent contrastive loss (N=4096, D=512, T=0.5).

v5: symmetry-exploiting variant. Global S = z z^T is symmetric, so each core
computes only its row block [0:1024) x a 5120-column window [0:5120) of its
ROTATED similarity matrix (rotation by 1024*c). Block-distance coverage:
  d0 (diag block) and d4 (antipodal block): every unordered pair appears as
     two computed entries across the fleet -> row-sums only.
  d1..d3: every unordered pair appears exactly once -> row-sum AND column-sum
     (the column-sum credits the transposed row's denominator; merged on host).
Norms are window-local: each core normalizes only the 5120 rows it uses as
columns (rows [0:5120) rotated), so there is no cross-core norm exchange.

Inputs per core (host-staged): xr = rotated rows [0:5120) bf16 row-major,
xt = rotated cols [0:5120) bf16 transposed. fp8e4m3 DoubleRow matmuls on
z8 = 16 * z (exp scale folds 1/256). Denominator parts: row-sums via fused
ACT-exp accumulation; column-sums for d1-d3 via ones-vector matmuls chained
over the 8 m-tiles in PSUM. Host merges rowsums [128,8], colsums [1,3072],
pair-dots [128,8]: loss = (sum log(den - e^2) - 2*sum pos) / 8192.
"""

import numpy as np
import ml_dtypes

for _p in ("/opt/trn_rl_repo", "/root/.axon_site/_ro/trn_rl_repo"):
    try:
        import concourse  # noqa: F401
        break
    except ImportError:
        import sys
        if _p not in sys.path:
            sys.path.insert(0, _p)

import concourse.bass as bass
import concourse.bacc as bacc
import concourse.tile as tile
from concourse import mybir
from concourse.bass_utils import run_bass_kernel_spmd

F32 = mybir.dt.float32
I32 = mybir.dt.int32
BF16 = mybir.dt.bfloat16
FP8 = mybir.dt.float8e4
ALU = mybir.AluOpType
AF = mybir.ActivationFunctionType

N_CORES = 8
BATCH = 4096
DIM = 512
ROWS = 2 * BATCH            # 8192
BLOCK = ROWS // N_CORES     # 1024 rows per core
P = 128                     # partitions
KC = DIM // P               # 4 k-chunks
MT = BLOCK // P             # 8 m-tiles
NCG = 5                     # column sub-groups in the window
CGW = 1024                  # cols per group
WIN = NCG * CGW             # 5120-column window
NW = 512                    # matmul free width
CS_LO, CS_HI = 1, 4         # groups with column-sum credit (d1..d3)
TEMP_SCALE = 2.0            # 1/T
SCALE_UP = 16.0             # fp8 pre-scale; exp scale folds 1/SCALE_UP^2
MAGIC = 0x5F3759DF


def _build_program():
    nc = bacc.Bacc(trn_type="TRN2")
    xr_in = nc.declare_dram_parameter("xr", [WIN, DIM], BF16, isOutput=False)
    xt_in = nc.declare_dram_parameter("xt", [DIM, WIN], BF16, isOutput=False)
    den_out = nc.declare_dram_parameter("den", [P, MT], F32, isOutput=True)
    pos_out = nc.declare_dram_parameter("pos", [P, MT], F32, isOutput=True)
    cs_out = nc.declare_dram_parameter(
        "cs", [1, (CS_HI - CS_LO) * CGW], F32, isOutput=True)

    with tile.TileContext(nc) as tc:
        with tc.tile_pool(name="xg", bufs=3) as xg_pool, \
             tc.tile_pool(name="sq", bufs=3) as sq_pool, \
             tc.tile_pool(name="small", bufs=2) as small_pool, \
             tc.tile_pool(name="ztr", bufs=8) as ztr_pool, \
             tc.tile_pool(name="bcast", bufs=2) as bcast_pool, \
             tc.tile_pool(name="single", bufs=1) as singles, \
             tc.tile_pool(name="escr", bufs=3) as e_pool, \
             tc.tile_pool(name="invd", bufs=2, space="DRAM") as invd_pool, \
             tc.tile_pool(name="mmps", bufs=3, space="PSUM") as mm_psum, \
             tc.tile_pool(name="csps", bufs=1, space="PSUM") as cs_psum:

            n2 = singles.tile([P, NCG * MT], F32, tag="n2")
            inv = singles.tile([P, NCG * MT], F32, tag="inv")
            magic = singles.tile([P, MT], I32, tag="magic")
            nc.vector.memset(magic, MAGIC)
            ones = singles.tile([P, 1], BF16, tag="ones")
            nc.vector.memset(ones, 1.0)
            accm = singles.tile([P, MT * NCG], F32, tag="accm")
            posraw = singles.tile([P, MT], F32, tag="posraw")
            pos_t = singles.tile([P, MT], F32, tag="pos_t")
            den_t = singles.tile([P, MT], F32, tag="den_t")
            colsb = singles.tile([1, (CS_HI - CS_LO) * CGW], F32, tag="colsb")
            # pinned rows 0-1023 (positive-pair partners for rows 4096-5119)
            xg_pin = singles.tile([P, MT, DIM], BF16, tag="xgpin")
            # scaled fp8 zT tiles, k-interleaved for DoubleRow:
            # zt8[kk][cg][p, i, c] = SCALE_UP * z[col c, d=256*kk+128*i+p]
            zt8 = [[singles.tile([P, 2, CGW], FP8, tag=f"zt8_{kk}_{c}",
                                 name=f"zt8_{kk}_{c}")
                    for c in range(NCG)] for kk in range(2)]

            def prep(cg):
                # --- load raw rows 1024cg..1024cg+1023, square-accum ---
                r0 = cg * CGW
                xg = xg_pin if cg == 0 else xg_pool.tile(
                    [P, MT, DIM], BF16, tag="xg")
                nc.sync.dma_start(
                    out=xg,
                    in_=xr_in[r0:r0 + CGW, :].rearrange(
                        "(a p) d -> p a d", p=P))
                for a in range(MT):
                    sq = sq_pool.tile([P, DIM], BF16, tag="sq")
                    nc.vector.scalar_tensor_tensor(
                        out=sq, in0=xg[:, a, :], scalar=0.0,
                        in1=xg[:, a, :], op0=ALU.bypass, op1=ALU.mult,
                        accum_out=n2[:, cg * MT + a: cg * MT + a + 1])
                if cg == 4:
                    # positive pairs: rotated rows [0:1024) x [4096:5120)
                    for a in range(MT):
                        psc = sq_pool.tile([P, DIM], BF16, tag="sq")
                        nc.vector.scalar_tensor_tensor(
                            out=psc, in0=xg_pin[:, a, :], scalar=0.0,
                            in1=xg[:, a, :], op0=ALU.bypass, op1=ALU.mult,
                            accum_out=posraw[:, a: a + 1])
                # --- rsqrt on this cg's 8 norms: Quake seed + 2 Newton ---
                sl = n2[:, cg * MT:(cg + 1) * MT]
                isl = inv[:, cg * MT:(cg + 1) * MT]
                sh = small_pool.tile([P, MT], I32, tag="sh")
                nc.vector.tensor_scalar(
                    out=sh, in0=sl.bitcast(I32), scalar1=1, scalar2=None,
                    op0=ALU.logical_shift_right)
                seed = small_pool.tile([P, MT], I32, tag="seed")
                nc.vector.scalar_tensor_tensor(
                    out=seed, in0=magic, scalar=0.0, in1=sh,
                    op0=ALU.bypass, op1=ALU.subtract)
                y = seed.bitcast(F32)
                for it in range(2):
                    ta = small_pool.tile([P, MT], F32, tag="ta")
                    tb = small_pool.tile([P, MT], F32, tag="tb")
                    nc.vector.tensor_mul(out=ta, in0=y, in1=y)
                    nc.vector.scalar_tensor_tensor(
                        out=tb, in0=ta, scalar=-0.5, in1=sl,
                        op0=ALU.mult, op1=ALU.mult)
                    nc.vector.tensor_scalar(
                        out=tb, in0=tb, scalar1=1.5, scalar2=None, op0=ALU.add)
                    dst = isl if it == 1 else y
                    nc.vector.tensor_mul(out=dst, in0=y, in1=tb)
                # inv * SCALE_UP as bf16 for the fp8 pre-scale
                iv16 = small_pool.tile([P, 32], BF16, tag="iv16")
                nc.vector.tensor_scalar(
                    out=iv16[:, 0:MT], in0=isl, scalar1=SCALE_UP, scalar2=None,
                    op0=ALU.mult)
                # --- inv -> row-ordered DRAM via DVE 32x32 block transpose,
                # then one contiguous partition-broadcast read back ---
                ivt = small_pool.tile([P, 32], BF16, tag="ivt")
                nc.vector.transpose(out=ivt, in_=iv16)
                invd = invd_pool.tile([CGW], BF16, tag="invd")
                for b in range(4):
                    nc.scalar.dma_start(
                        out=invd.rearrange("(i b j) -> i b j", b=4, j=32)
                                [:, b, :],
                        in_=ivt[32 * b:32 * b + MT, :])
                bc = bcast_pool.tile([P, CGW], BF16, tag="bc")
                nc.scalar.dma_start(
                    out=bc,
                    in_=invd.rearrange("(a f) -> a f", a=1)
                            .partition_broadcast(P))
                # --- load transposed tiles, scale into fp8 DoubleRow layout ---
                for k in range(KC):
                    ztr = ztr_pool.tile([P, CGW], BF16, tag="ztr")
                    nc.sync.dma_start(
                        out=ztr,
                        in_=xt_in[k * P:(k + 1) * P,
                                  cg * CGW:(cg + 1) * CGW])
                    nc.vector.tensor_mul(
                        out=zt8[k // 2][cg][:, k % 2, :], in0=ztr, in1=bc)

            def mmblock(cg):
                do_cs = CS_LO <= cg < CS_HI
                cs = (cs_psum.tile([1, CGW], F32, tag="cs", name=f"cs{cg}")
                      if do_cs else None)
                es = []
                for m in range(MT):
                    ps = mm_psum.tile([P, CGW], F32, tag="ps")
                    for n in range(CGW // NW):
                        for kk in range(2):
                            nc.tensor.matmul(
                                ps[:, n * NW:(n + 1) * NW],
                                lhsT=zt8[kk][0][:, :, m * P:(m + 1) * P],
                                rhs=zt8[kk][cg][:, :, n * NW:(n + 1) * NW],
                                start=(kk == 0), stop=(kk == 1),
                                perf_mode=mybir.MatmulPerfMode.DoubleRow)
                    e_scr = e_pool.tile([P, CGW], BF16, tag="escr")
                    nc.scalar.activation(
                        out=e_scr, in_=ps, func=AF.Exp,
                        scale=TEMP_SCALE / (SCALE_UP * SCALE_UP),
                        accum_out=accm[:, m * NCG + cg: m * NCG + cg + 1])
                    if do_cs:
                        es.append((m, e_scr))
                        # column sums: ones^T @ e, chained over m in PSUM.
                        # Emit one m behind so the PE never waits on exp.
                        if m >= 1:
                            pm, pe = es[m - 1]
                            for n in range(CGW // NW):
                                nc.tensor.matmul(
                                    cs[:, n * NW:(n + 1) * NW],
                                    lhsT=ones, rhs=pe[:, n * NW:(n + 1) * NW],
                                    start=(pm == 0), stop=False,
                                    skip_group_check=True)
                if do_cs:
                    pm, pe = es[MT - 1]
                    for n in range(CGW // NW):
                        nc.tensor.matmul(
                            cs[:, n * NW:(n + 1) * NW],
                            lhsT=ones, rhs=pe[:, n * NW:(n + 1) * NW],
                            start=False, stop=True, skip_group_check=True)
                    nc.vector.tensor_copy(
                        out=colsb[:, (cg - CS_LO) * CGW:
                                  (cg - CS_LO + 1) * CGW], in_=cs)

            prep(0)
            prep(1)
            for cg in range(NCG):
                mmblock(cg)
                if cg + 2 < NCG:
                    prep(cg + 2)

            # --- outputs: raw denominator row-sums + scaled pair-dots ---
            for m in range(MT):
                nc.vector.reduce_sum(
                    out=den_t[:, m:m + 1], in_=accm[:, m * NCG:(m + 1) * NCG],
                    axis=mybir.AxisListType.X)
            nc.vector.tensor_mul(out=pos_t, in0=posraw, in1=inv[:, 0:MT])
            nc.vector.tensor_mul(
                out=pos_t, in0=pos_t, in1=inv[:, 4 * MT:5 * MT])
            nc.sync.dma_start(out=den_out[:, :], in_=den_t)
            nc.sync.dma_start(out=pos_out[:, :], in_=pos_t)
            nc.sync.dma_start(out=cs_out[:, :], in_=colsb)

    nc.finalize()
    return nc


_CACHE = {}


def _run(full: np.ndarray, trace: bool = False, **kwargs):
    """Run the SPMD program on all 8 cores; returns BassKernelResults."""
    if "nc" not in _CACHE:
        _CACHE["nc"] = _build_program()
    nc = _CACHE["nc"]
    xbf = full.astype(ml_dtypes.bfloat16)
    in_maps = []
    for c in range(N_CORES):
        xc = np.ascontiguousarray(np.roll(xbf, -BLOCK * c, axis=0)[:WIN])
        in_maps.append({
            "xr": xc,
            "xt": np.ascontiguousarray(xc.T),
        })
    return run_bass_kernel_spmd(
        nc, in_maps, core_ids=list(range(N_CORES)), trace=trace, **kwargs)


def _merge(results) -> np.ndarray:
    e2 = float(np.exp(2.0))
    den_g = np.zeros(ROWS, dtype=np.float64)
    pos_sum = 0.0
    for c, r in enumerate(results):
        # row-sums: (p, m) -> rotated row 128m+p -> global 1024c+128m+p
        den = r["den"].astype(np.float64)          # [128, 8]
        rows = (1024 * c + 128 * np.arange(MT)[None, :]
                + np.arange(P)[:, None]) % ROWS
        np.add.at(den_g, rows, den)
        # col-sums: j in [0,3072) -> rotated col 1024+j -> global row
        cs = r["cs"].astype(np.float64).ravel()    # [3072]
        cols = (1024 * c + CGW * CS_LO + np.arange(cs.size)) % ROWS
        np.add.at(den_g, cols, cs)
        pos_sum += r["pos"].astype(np.float64).sum()
    loss = (np.log(den_g - e2).sum() - TEMP_SCALE * pos_sum) / (2.0 * BATCH)
    return np.array(loss, dtype=np.float32)


def kernel(emb_i: np.ndarray, emb_j: np.ndarray) -> np.ndarray:
    full = np.concatenate(
        [np.asarray(emb_i, np.float32), np.asarray(emb_j, np.float32)], axis=0)
    return _merge(_run(full).results)
